# revision 17
# baseline (speedup 1.0000x reference)
"""Trainium2 Bass kernel for nn_Graph_to_Featuremaps_savemem.

Math: softmax over nodes is shift-invariant, so the (res @ nfr)[b,p] term
cancels and res_feature never affects the output:
    attn[b,p,:] = softmax(x[b] @ nfh)          (independent of p)
    out[b,c,h,w] = relu(((e_b^T x[b]) @ W)[c] / sum(e_b))   broadcast over (h,w)
with e_b = exp(x[b] @ nfh). The kernel is a tiny per-batch softmax-weighted
reduction followed by a huge broadcast write — pure HBM-write-bound, sharded
batch-parallel over 8 cores (2 batches/core).

Performance structure (per core):
  - Output is written in float16 (host upcasts): 16 MB instead of 32 MB.
    fp16 quantization adds ~3e-4 rms rel err, far inside the 2e-2 gate.
  - Inputs arrive as two packed bf16 DRAM buffers (pa: X^T|nfh on the sync
    ring — the critical path; pb: X|W on the scalar ring). X^T is transposed
    on host, removing the on-device PE transpose from the critical path.
  - All matmuls run on bf16 inputs: single pass, fp32 PSUM accumulation.
  - Fill tiles are built by ACT and DVE in parallel from [128,1]
    VR = relu(V/sum) columns (separate copies per engine so no cross-engine
    ordering can appear). DVE fills read a GpSimd-memset ZERO tile
    sequentially (1.0us) instead of a stride-0 broadcast source (1.75us).
  - Output rides plain column-range DMAs with 2-8 KB descriptors on the
    otherwise-idle sync ring. (A stride-0 repeat-AP source was tried and
    writes at full rate on 15 engines but triggers periodic ~+200ns packet
    stalls on SDMA engine 15, costing ~7us — avoid.)
  - Block 0 starts with a narrow 1024-col fill so the first DMA issues
    ~1.5us earlier; the critical chain is wrapped in tc.high_priority() so
    the Tile scheduler doesn't push it behind off-path work.
"""

import numpy as np

N_CORES = 8
B, NODES, HID, C, H, W = 16, 64, 128, 256, 128, 128
HWP = H * W  # 16384
B_LOC = B // N_CORES  # 2 batches per core
FILL_F = 4096  # fill tile free width for blocks 1..3 (8 KB descriptors)
F0A = 1024  # block-0 fast-start fill width
F0B = 3072  # block-0 main fill width (5 column-range DMAs re-read it)
ACT_W = 1024  # columns of each block-1..3 fill computed by ACT (rest: DVE)
PA_COLS = 256  # XT(128) | nfh(1) | pad -> 512B/partition descriptors
PB_COLS = 384  # X(128) | W(256)

_NC_CACHE = {}


def build_nc():
    import concourse.bass as bass
    import concourse.bacc as bacc
    import concourse.mybir as mybir
    from concourse.tile import TileContext

    f32 = mybir.dt.float32
    bf16 = mybir.dt.bfloat16
    f16 = mybir.dt.float16
    Alu = mybir.AluOpType
    Act = mybir.ActivationFunctionType

    nc = bacc.Bacc(None, target_bir_lowering=False, debug=False)
    pa_d = nc.declare_dram_parameter("pa", [128, PA_COLS], bf16, isOutput=False)
    pb_d = nc.declare_dram_parameter("pb", [128, PB_COLS], bf16, isOutput=False)
    out_d = nc.declare_dram_parameter("out", [B_LOC * C, HWP], f16, isOutput=True)

    def bcast(ap, n):
        # (P,1) AP -> (P,n) AP re-reading the same element along free dim
        return type(ap)(ap.tensor, ap.offset, [list(ap.ap[0]), [0, n]])

    with TileContext(nc) as tc:
        with (
            nc.allow_low_precision(reason="fp16 output within 2e-2 rel-err gate"),
            tc.tile_pool(name="singles", bufs=1) as singles,
            tc.tile_pool(name="fills", bufs=1) as fills,
            tc.tile_pool(name="psum", bufs=4, space="PSUM") as psum,
            tc.tile_pool(name="psumv", bufs=1, space="PSUM") as psumv,
        ):
            # ---- constants (no input deps, overlap the input DMAs) ----
            MASK2 = singles.tile([128, 2], bf16, tag="MASK2")
            nc.vector.memset(MASK2[:], 0.0)
            nc.vector.memset(MASK2[0:64, 0:1], 1.0)
            nc.vector.memset(MASK2[64:128, 1:2], 1.0)
            ONES1 = singles.tile([1, 128], bf16, tag="ONES1")
            nc.vector.memset(ONES1[:], 1.0)
            ZERO = singles.tile([128, FILL_F - ACT_W], f16, tag="ZERO")
            nc.gpsimd.memset(ZERO[:], 0.0)

            # ---- packed input loads (pa on sync ring, pb on scalar ring) ----
            PA = singles.tile([128, PA_COLS], bf16, tag="PA")
            nc.sync.dma_start(out=PA[:], in_=pa_d[:])
            PB = singles.tile([128, PB_COLS], bf16, tag="PB")
            nc.scalar.dma_start(out=PB[:], in_=pb_d[:])

            XT = PA[:, 0:HID]
            NFH = PA[:, HID : HID + 1]
            X = PB[:, 0:HID]
            Wt = PB[:, HID : HID + C]

            # ---- critical chain: s = X @ nfh, e = exp(s), per-batch sums,
            #      reciprocals broadcast to RC[:, b] = 1/sum_b ----
            with tc.high_priority():
                s_ps = psum.tile([128, 1], f32, tag="ps")
                nc.tensor.matmul(s_ps[:], XT, NFH)
                e_col = singles.tile([128, 1], bf16, tag="e_col")
                nc.scalar.activation(e_col[:], s_ps[:], Act.Exp)

                S2_ps = psum.tile([1, 2], f32, tag="ps")
                nc.tensor.matmul(S2_ps[:], e_col[:], MASK2[:])
                r_row = singles.tile([1, 2], bf16, tag="r_row")
                nc.vector.reciprocal(r_row[:], S2_ps[:])
                RC_ps = psum.tile([128, 2], f32, tag="ps")
                nc.tensor.matmul(RC_ps[:], ONES1[:], r_row[:])
                RC = singles.tile([128, 2], f32, tag="RC")
                nc.vector.tensor_copy(RC[:], RC_ps[:])

            # U'[b] = X[b]^T @ e[b]
            U_ps = [
                psum.tile([HID, 1], f32, tag="ps", name=f"U_ps{b}")
                for b in range(B_LOC)
            ]
            U_sb = [
                singles.tile([HID, 1], bf16, tag=f"U_sb{b}", name=f"U_sb{b}")
                for b in range(B_LOC)
            ]
            sl0 = slice(0, NODES)
            nc.tensor.matmul(U_ps[0][:], X[sl0, :], e_col[sl0, :])
            nc.scalar.activation(U_sb[0][:], U_ps[0][:], Act.Copy)

            def make_v(b, hf):
                V_ps = psumv.tile(
                    [128, 1], f32, tag=f"V_ps{b}{hf}", name=f"V_ps{b}{hf}"
                )
                nc.tensor.matmul(
                    V_ps[:], Wt[:, hf * 128 : (hf + 1) * 128], U_sb[b][:]
                )
                return V_ps

            def make_vr(V_ps, b, hf, suffix):
                VR = singles.tile(
                    [128, 1], f32, tag=f"VR{suffix}{b}{hf}", name=f"VR{suffix}{b}{hf}"
                )
                nc.vector.tensor_scalar(
                    VR[:], V_ps[:], RC[:, b : b + 1], 0.0,
                    op0=Alu.mult, op1=Alu.max,
                )
                return VR

            # ---- block 0 (b=0, hf=0): fast-start narrow fill + main fill ----
            with tc.high_priority():
                V00 = make_v(0, 0)
                VRd00 = make_vr(V00, 0, 0, "d")
                f0a = fills.tile([128, F0A], f16, tag="f0a")
                nc.vector.tensor_scalar(
                    f0a[:], ZERO[:, 0:F0A], VRd00[:], 0.0, op0=Alu.add, op1=Alu.max
                )
                nc.sync.dma_start(out=out_d[0:128, 0:F0A], in_=f0a[:])
                f0b = fills.tile([128, F0B], f16, tag="f0b")
                nc.vector.tensor_scalar(
                    f0b[:], ZERO[:, 0:F0B], VRd00[:], 0.0, op0=Alu.add, op1=Alu.max
                )
                for s in range(5):
                    lo = F0A + s * F0B
                    nc.sync.dma_start(
                        out=out_d[0:128, lo : lo + F0B], in_=f0b[:]
                    )

            # ---- blocks 1..3: [128, FILL_F] fills split ACT | DVE ----
            def emit_block(b, hf):
                V_ps = make_v(b, hf)
                fill = fills.tile(
                    [128, FILL_F], f16, tag=f"fill{b}{hf}", name=f"fill{b}{hf}"
                )
                VRa = make_vr(V_ps, b, hf, "a")
                nc.scalar.activation(
                    fill[:, 0:ACT_W], bcast(VRa[:], ACT_W), Act.Copy
                )
                VRd = make_vr(V_ps, b, hf, "d")
                nc.vector.tensor_scalar(
                    fill[:, ACT_W:FILL_F], ZERO[:], VRd[:], 0.0,
                    op0=Alu.add, op1=Alu.max,
                )
                r0 = b * C + hf * 128
                for s in range(HWP // FILL_F):
                    nc.sync.dma_start(
                        out=out_d[r0 : r0 + 128, s * FILL_F : (s + 1) * FILL_F],
                        in_=fill[:],
                    )

            emit_block(0, 1)
            sl1 = slice(NODES, 2 * NODES)
            nc.tensor.matmul(U_ps[1][:], X[sl1, :], e_col[sl1, :])
            nc.scalar.activation(U_sb[1][:], U_ps[1][:], Act.Copy)
            emit_block(1, 0)
            emit_block(1, 1)
    nc.finalize()
    return nc


def get_nc():
    if "nc" not in _NC_CACHE:
        _NC_CACHE["nc"] = build_nc()
    return _NC_CACHE["nc"]


def make_in_maps(input, node_fea_for_hidden, weight):
    import ml_dtypes

    bf = ml_dtypes.bfloat16
    x = np.asarray(input, np.float32)[0]  # (B, NODES, HID)
    nfh = np.asarray(node_fea_for_hidden, np.float32).reshape(HID)
    w = np.asarray(weight, np.float32)  # (HID, C)
    in_maps = []
    for i in range(N_CORES):
        xs = x[i * B_LOC : (i + 1) * B_LOC].reshape(B_LOC * NODES, HID)
        pa = np.zeros((128, PA_COLS), bf)
        pa[:, 0:HID] = xs.T.astype(bf)
        pa[:, HID] = nfh.astype(bf)
        pb = np.empty((128, PB_COLS), bf)
        pb[:, 0:HID] = xs.astype(bf)
        pb[:, HID:] = w.astype(bf)
        in_maps.append(
            {"pa": np.ascontiguousarray(pa), "pb": np.ascontiguousarray(pb)}
        )
    return in_maps


def run_spmd(in_maps, trace=False, **kw):
    from concourse.bass_utils import run_bass_kernel_spmd

    return run_bass_kernel_spmd(get_nc(), in_maps, list(range(N_CORES)), trace=trace, **kw)


def kernel(input, res_feature, node_fea_for_res, node_fea_for_hidden, weight):
    res = run_spmd(make_in_maps(input, node_fea_for_hidden, weight)).results
    out = np.concatenate(
        [r["out"].reshape(B_LOC, C, H, W) for r in res], axis=0
    )
    return out.astype(np.float32)


# revision 18
# speedup vs baseline: 1.0106x; 1.0106x over previous
"""Trainium2 Bass kernel for nn_Graph_to_Featuremaps_savemem.

Math: softmax over nodes is shift-invariant, so the (res @ nfr)[b,p] term
cancels and res_feature never affects the output:
    attn[b,p,:] = softmax(x[b] @ nfh)          (independent of p)
    out[b,c,h,w] = relu(((e_b^T x[b]) @ W)[c] / sum(e_b))   broadcast over (h,w)
with e_b = exp(x[b] @ nfh). The kernel is a tiny per-batch softmax-weighted
reduction followed by a huge broadcast write — pure HBM-write-bound, sharded
batch-parallel over 8 cores (2 batches/core).

Performance structure (per core):
  - Output is written in float16 (host upcasts): 16 MB instead of 32 MB.
    fp16 quantization adds ~3e-4 rms rel err, far inside the 2e-2 gate.
  - Inputs arrive as two packed bf16 DRAM buffers (pa: X^T|nfh on the sync
    ring — the critical path; pb: X|W on the scalar ring). X^T is transposed
    on host, removing the on-device PE transpose from the critical path.
  - All matmuls run on bf16 inputs: single pass, fp32 PSUM accumulation.
  - Fill tiles [128, 4096] f16 are built by ACT (bcast-copy of a [128,1]
    VR = relu(V/sum) column) and DVE (tensor_scalar over a memset ZERO tile —
    sequential reads, 1.0us, vs 1.75us for a stride-0 broadcast read) in
    parallel, with per-engine private VR copies so no cross-engine ordering
    can appear. The critical chain is wrapped in tc.high_priority().
  - Output: 16 plain column-range DMAs with UNIFORM 8 KB descriptors on the
    otherwise-idle sync ring. Empirically, SDMA engine 15 develops periodic
    ~+200ns/packet stalls (~+7us tail) whenever the descriptor stream mixes
    in small (<=6KB) descriptors, a repeat-AP source, or a concurrent GpSimd
    memset runs — all three are avoided deliberately.
"""

import numpy as np

N_CORES = 8
B, NODES, HID, C, H, W = 16, 64, 128, 256, 128, 128
HWP = H * W  # 16384
B_LOC = B // N_CORES  # 2 batches per core
FILL_F = 4096  # fill tile free width (8 KB descriptors, 4 DMAs per block)
ACT_W = 768  # columns of each fill computed by ACT (rest: DVE), latency-balanced
PA_COLS = 256  # XT(128) | nfh(1) | pad -> 512B/partition descriptors
PB_COLS = 384  # X(128) | W(256)

_NC_CACHE = {}


def build_nc():
    import concourse.bass as bass
    import concourse.bacc as bacc
    import concourse.mybir as mybir
    from concourse.tile import TileContext

    f32 = mybir.dt.float32
    bf16 = mybir.dt.bfloat16
    f16 = mybir.dt.float16
    Alu = mybir.AluOpType
    Act = mybir.ActivationFunctionType

    nc = bacc.Bacc(None, target_bir_lowering=False, debug=False)
    pa_d = nc.declare_dram_parameter("pa", [128, PA_COLS], bf16, isOutput=False)
    pb_d = nc.declare_dram_parameter("pb", [128, PB_COLS], bf16, isOutput=False)
    out_d = nc.declare_dram_parameter("out", [B_LOC * C, HWP], f16, isOutput=True)

    def bcast(ap, n):
        # (P,1) AP -> (P,n) AP re-reading the same element along free dim
        return type(ap)(ap.tensor, ap.offset, [list(ap.ap[0]), [0, n]])

    with TileContext(nc) as tc:
        with (
            nc.allow_low_precision(reason="fp16 output within 2e-2 rel-err gate"),
            tc.tile_pool(name="singles", bufs=1) as singles,
            tc.tile_pool(name="fills", bufs=1) as fills,
            tc.tile_pool(name="psum", bufs=4, space="PSUM") as psum,
            tc.tile_pool(name="psumv", bufs=1, space="PSUM") as psumv,
        ):
            # ---- constants (DVE, overlap the input DMAs; MASK2 first since
            #      S2 needs it earliest, ZERO's 2.6us fits before recip) ----
            MASK2 = singles.tile([128, 2], bf16, tag="MASK2")
            nc.vector.memset(MASK2[:], 0.0)
            nc.vector.memset(MASK2[0:64, 0:1], 1.0)
            nc.vector.memset(MASK2[64:128, 1:2], 1.0)
            ONES1 = singles.tile([1, 128], bf16, tag="ONES1")
            nc.vector.memset(ONES1[:], 1.0)
            ZERO = singles.tile([128, FILL_F - ACT_W], f16, tag="ZERO")
            nc.vector.memset(ZERO[:], 0.0)

            # ---- packed input loads (pa on sync ring, pb on scalar ring) ----
            PA = singles.tile([128, PA_COLS], bf16, tag="PA")
            nc.sync.dma_start(out=PA[:], in_=pa_d[:])
            PB = singles.tile([128, PB_COLS], bf16, tag="PB")
            nc.scalar.dma_start(out=PB[:], in_=pb_d[:])

            XT = PA[:, 0:HID]
            NFH = PA[:, HID : HID + 1]
            X = PB[:, 0:HID]
            Wt = PB[:, HID : HID + C]

            # ---- critical chain: s = X @ nfh, e = exp(s), per-batch sums,
            #      reciprocals broadcast to RC[:, b] = 1/sum_b ----
            with tc.high_priority():
                s_ps = psum.tile([128, 1], f32, tag="ps")
                nc.tensor.matmul(s_ps[:], XT, NFH)
                e_col = singles.tile([128, 1], bf16, tag="e_col")
                nc.scalar.activation(e_col[:], s_ps[:], Act.Exp)

                S2_ps = psum.tile([1, 2], f32, tag="ps")
                nc.tensor.matmul(S2_ps[:], e_col[:], MASK2[:])
                r_row = singles.tile([1, 2], bf16, tag="r_row")
                nc.vector.reciprocal(r_row[:], S2_ps[:])
                RC_ps = psum.tile([128, 2], f32, tag="ps")
                nc.tensor.matmul(RC_ps[:], ONES1[:], r_row[:])
                RC = singles.tile([128, 2], f32, tag="RC")
                nc.vector.tensor_copy(RC[:], RC_ps[:])

            # U'[b] = X[b]^T @ e[b]
            U_ps = [
                psum.tile([HID, 1], f32, tag="ps", name=f"U_ps{b}")
                for b in range(B_LOC)
            ]
            U_sb = [
                singles.tile([HID, 1], bf16, tag=f"U_sb{b}", name=f"U_sb{b}")
                for b in range(B_LOC)
            ]

            def emit_block(b, hf, prio):
                V_ps = psumv.tile(
                    [128, 1], f32, tag=f"V_ps{b}{hf}", name=f"V_ps{b}{hf}"
                )
                nc.tensor.matmul(
                    V_ps[:], Wt[:, hf * 128 : (hf + 1) * 128], U_sb[b][:]
                )
                fill = fills.tile(
                    [128, FILL_F], f16, tag=f"fill{b}{hf}", name=f"fill{b}{hf}"
                )
                # VR* = max(V/sum, 0); private per consumer engine
                VRa = singles.tile(
                    [128, 1], f32, tag=f"VRa{b}{hf}", name=f"VRa{b}{hf}"
                )
                nc.vector.tensor_scalar(
                    VRa[:], V_ps[:], RC[:, b : b + 1], 0.0,
                    op0=Alu.mult, op1=Alu.max,
                )
                VRd = singles.tile(
                    [128, 1], f32, tag=f"VRd{b}{hf}", name=f"VRd{b}{hf}"
                )
                nc.vector.tensor_scalar(
                    VRd[:], V_ps[:], RC[:, b : b + 1], 0.0,
                    op0=Alu.mult, op1=Alu.max,
                )
                nc.scalar.activation(
                    fill[:, 0:ACT_W], bcast(VRa[:], ACT_W), Act.Copy
                )
                nc.vector.tensor_scalar(
                    fill[:, ACT_W:FILL_F], ZERO[:], VRd[:], 0.0,
                    op0=Alu.add, op1=Alu.max,
                )
                r0 = b * C + hf * 128
                for s in range(HWP // FILL_F):
                    nc.sync.dma_start(
                        out=out_d[r0 : r0 + 128, s * FILL_F : (s + 1) * FILL_F],
                        in_=fill[:],
                    )

            sl0 = slice(0, NODES)
            with tc.high_priority():
                nc.tensor.matmul(U_ps[0][:], X[sl0, :], e_col[sl0, :])
                nc.scalar.activation(U_sb[0][:], U_ps[0][:], Act.Copy)
                emit_block(0, 0, True)
            emit_block(0, 1, False)
            sl1 = slice(NODES, 2 * NODES)
            nc.tensor.matmul(U_ps[1][:], X[sl1, :], e_col[sl1, :])
            nc.scalar.activation(U_sb[1][:], U_ps[1][:], Act.Copy)
            emit_block(1, 0, False)
            emit_block(1, 1, False)
    nc.finalize()
    return nc


def get_nc():
    if "nc" not in _NC_CACHE:
        _NC_CACHE["nc"] = build_nc()
    return _NC_CACHE["nc"]


def make_in_maps(input, node_fea_for_hidden, weight):
    import ml_dtypes

    bf = ml_dtypes.bfloat16
    x = np.asarray(input, np.float32)[0]  # (B, NODES, HID)
    nfh = np.asarray(node_fea_for_hidden, np.float32).reshape(HID)
    w = np.asarray(weight, np.float32)  # (HID, C)
    in_maps = []
    for i in range(N_CORES):
        xs = x[i * B_LOC : (i + 1) * B_LOC].reshape(B_LOC * NODES, HID)
        pa = np.zeros((128, PA_COLS), bf)
        pa[:, 0:HID] = xs.T.astype(bf)
        pa[:, HID] = nfh.astype(bf)
        pb = np.empty((128, PB_COLS), bf)
        pb[:, 0:HID] = xs.astype(bf)
        pb[:, HID:] = w.astype(bf)
        in_maps.append(
            {"pa": np.ascontiguousarray(pa), "pb": np.ascontiguousarray(pb)}
        )
    return in_maps


def run_spmd(in_maps, trace=False, **kw):
    from concourse.bass_utils import run_bass_kernel_spmd

    return run_bass_kernel_spmd(get_nc(), in_maps, list(range(N_CORES)), trace=trace, **kw)


def kernel(input, res_feature, node_fea_for_res, node_fea_for_hidden, weight):
    res = run_spmd(make_in_maps(input, node_fea_for_hidden, weight)).results
    out = np.concatenate(
        [r["out"].reshape(B_LOC, C, H, W) for r in res], axis=0
    )
    return out.astype(np.float32)
